# revision 7
# baseline (speedup 1.0000x reference)
"""MoE ConditionalFeedForward (SwiGLU expert FFN) for 8 Trainium2 NeuronCores.

Expert-parallel: core e holds expert e's weights (host pre-transposed) and
computes the full SwiGLU FFN for ALL 16 tokens densely:
    y_e = (silu(x @ w1[e].T) * (x @ w3[e].T)) @ w2[e].T        [16, 1024]
The (token, slot) -> expert routing is a pure gather done on the host:
    out[t, a] = y_{expert_indices[t, a]}[t]
Since T=16 <= 128, computing all tokens per expert costs the same PE time as
computing only the routed ones, and weight DMA (34.6 MB/core) dominates.

All matmuls stream the weight matrix as the moving operand (N=512 free dim)
with the small [128, 16] activation tile stationary, so the PE streams each
weight element exactly once.  fp32 weights are fed as float32r (full-rate for
moving dim >= 256).
"""

import os
import threading

import numpy as np

NUM_EXPERTS = 8
INTER = 2816
DIM = 1024
T = 16
A = 2
P = 128
N_CORES = 8
KD = DIM // P  # 8 contraction chunks for stage 1
KI = INTER // P  # 22 contraction chunks for stage 2
J_TILES = [(0, 512), (512, 512), (1024, 512), (1536, 512), (2048, 512), (2560, 256)]

# "f32" (float32 weights, float32r matmuls) or "bf16" (bf16 weights/matmuls)
WEIGHT_MODE = os.environ.get("KERNEL_WEIGHT_MODE", "f32")

_lock = threading.Lock()
_nc_cache = {}
LAST_RESULTS = None  # BassKernelResults of the most recent kernel() call


def _build_nc(mode: str):
    import concourse.bass as bass
    import concourse.tile as tile
    from concourse import bacc, mybir

    f32 = mybir.dt.float32
    if mode == "f32":
        # float32r: same 4-byte layout as f32, full-rate PE matmul for
        # moving dim >= 256. The BIR verifier requires every matmul
        # operand to be *declared* f32r end-to-end, so DRAM params and
        # SBUF tiles all use it (numpy side stays np.float32).
        wdt = mybir.dt.float32r
    else:
        wdt = mybir.dt.bfloat16

    def mm_ap(ap):
        return ap

    # Bacc (not plain Bass): its compile() runs move_matmul_waits_to_ldweights
    # + generate_event_semaphores, required because fp32r matmuls lower to a
    # self-loading LDWEIGHTS struct with a single sync-wait slot.
    nc = bacc.Bacc()
    xt_d = nc.declare_dram_parameter("xt", [P, KD, T], wdt, isOutput=False)
    w1_d = nc.declare_dram_parameter("w1t", [DIM, INTER], wdt, isOutput=False)
    w3_d = nc.declare_dram_parameter("w3t", [DIM, INTER], wdt, isOutput=False)
    w2_d = nc.declare_dram_parameter("w2t", [INTER, DIM], wdt, isOutput=False)
    eye_d = nc.declare_dram_parameter("eye", [T, T], f32, isOutput=False)
    out_d = nc.declare_dram_parameter("out", [T, DIM], f32, isOutput=True)

    with tile.TileContext(nc) as tc:
        with (
            tc.tile_pool(name="const", bufs=1) as cpool,
            tc.tile_pool(name="w1p", bufs=3) as w1p,
            tc.tile_pool(name="w3p", bufs=3) as w3p,
            tc.tile_pool(name="w2p", bufs=3) as w2p,
            tc.tile_pool(name="hp", bufs=1) as hp,
            tc.tile_pool(name="ep", bufs=3) as ep,
            tc.tile_pool(name="outp", bufs=1) as outp,
            tc.tile_pool(name="ps1", bufs=4, space="PSUM") as ps1p,
            tc.tile_pool(name="pst", bufs=2, space="PSUM") as pstp,
            tc.tile_pool(name="pso", bufs=1, space="PSUM") as psop,
        ):
            xt_sb = cpool.tile([P, KD, T], wdt)
            nc.sync.dma_start(xt_sb[:], xt_d[:])
            eye_sb = cpool.tile([T, T], f32)
            nc.sync.dma_start(eye_sb[:], eye_d[:])

            h_sb = hp.tile([T, INTER], f32)
            hT_sb = hp.tile([P, KI * T], wdt)

            # ---- stage 1: x1 = x@w1.T, x3 = x@w3.T, h = silu(x1)*x3 ----
            w1_r = w1_d.rearrange("(k p) c -> p k c", p=P)  # [128, 8, 2816]
            w3_r = w3_d.rearrange("(k p) c -> p k c", p=P)
            for joff, jsz in J_TILES:
                w1b = w1p.tile([P, KD, jsz], wdt, tag="w1b")
                nc.sync.dma_start(w1b[:], w1_r[:, :, joff : joff + jsz])
                w3b = w3p.tile([P, KD, jsz], wdt, tag="w3b")
                nc.sync.dma_start(w3b[:], w3_r[:, :, joff : joff + jsz])

                ps1 = ps1p.tile([T, jsz], f32, tag="psa")
                ps3 = ps1p.tile([T, jsz], f32, tag="psa")
                for k in range(KD):
                    lhs = mm_ap(xt_sb[:, k, :])
                    nc.tensor.matmul(
                        ps1[:], lhs, mm_ap(w1b[:, k, :]),
                        start=(k == 0), stop=(k == KD - 1),
                    )
                for k in range(KD):
                    lhs = mm_ap(xt_sb[:, k, :])
                    nc.tensor.matmul(
                        ps3[:], lhs, mm_ap(w3b[:, k, :]),
                        start=(k == 0), stop=(k == KD - 1),
                    )

                # silu(x1)*x3 = x1*sigmoid(x1)*x3 (no Silu LUT on trn2 ACT)
                sg = ep.tile([T, jsz], f32, tag="sg")
                nc.scalar.activation(
                    sg[:], ps1[:], mybir.ActivationFunctionType.Sigmoid
                )
                tmp = ep.tile([T, jsz], f32, tag="tmp")
                nc.vector.tensor_mul(tmp[:], sg[:], ps3[:])
                nc.vector.tensor_mul(h_sb[:, joff : joff + jsz], tmp[:], ps1[:])

                # transpose the freshly finished h columns into [INTER, T]
                for c in range(joff // P, (joff + jsz) // P):
                    pt = pstp.tile([P, T], f32, tag="pt")
                    nc.tensor.transpose(
                        pt[:], h_sb[:, c * P : (c + 1) * P], eye_sb[:]
                    )
                    nc.vector.tensor_copy(hT_sb[:, c * T : (c + 1) * T], pt[:])

            # ---- stage 2: out = h @ w2.T ----
            w2_r = w2_d.rearrange("(g k p) c -> g p k c", k=2, p=P)  # [11,128,2,1024]
            pso0 = psop.tile([T, 512], f32)
            pso1 = psop.tile([T, 512], f32)
            psos = [pso0, pso1]
            for g in range(KI // 2):
                w2b = w2p.tile([P, 2, DIM], wdt, tag="w2b")
                nc.sync.dma_start(w2b[:], w2_r[g])
                for k in range(2):
                    ic = 2 * g + k
                    lhs = mm_ap(hT_sb[:, ic * T : (ic + 1) * T])
                    for d in range(2):
                        nc.tensor.matmul(
                            psos[d][:], lhs, mm_ap(w2b[:, k, d * 512 : (d + 1) * 512]),
                            start=(ic == 0), stop=(ic == KI - 1),
                        )

            out_sb = outp.tile([T, DIM], f32)
            nc.vector.tensor_copy(out_sb[:, 0:512], pso0[:])
            nc.vector.tensor_copy(out_sb[:, 512:1024], pso1[:])
            nc.sync.dma_start(out_d[:], out_sb[:])

    nc.compile()
    return nc


def _get_nc(mode: str):
    with _lock:
        if mode not in _nc_cache:
            _nc_cache[mode] = _build_nc(mode)
        return _nc_cache[mode]


def kernel(**inputs: np.ndarray) -> np.ndarray:
    global LAST_RESULTS
    from concourse.bass_utils import run_bass_kernel_spmd

    mode = WEIGHT_MODE
    x = np.asarray(inputs["x"], dtype=np.float32)
    expert_indices = np.asarray(inputs["expert_indices"]).astype(np.int64)
    w1 = np.asarray(inputs["w1"], dtype=np.float32)
    w2 = np.asarray(inputs["w2"], dtype=np.float32)
    w3 = np.asarray(inputs["w3"], dtype=np.float32)

    np_wdt = np.float32 if mode == "f32" else None
    if np_wdt is None:
        import ml_dtypes

        np_wdt = ml_dtypes.bfloat16

    # x.T laid out [128 partitions, 8 k-chunks, 16 tokens]
    xt = np.ascontiguousarray(
        x.T.reshape(KD, P, T).transpose(1, 0, 2), dtype=np_wdt
    )
    eye = np.eye(T, dtype=np.float32)

    in_maps = []
    for e in range(N_CORES):
        in_maps.append(
            {
                "xt": xt,
                "w1t": np.ascontiguousarray(w1[e].T, dtype=np_wdt),
                "w3t": np.ascontiguousarray(w3[e].T, dtype=np_wdt),
                "w2t": np.ascontiguousarray(w2[e].T, dtype=np_wdt),
                "eye": eye,
            }
        )

    nc = _get_nc(mode)
    res = run_bass_kernel_spmd(nc, in_maps, core_ids=list(range(N_CORES)))
    LAST_RESULTS = res

    y = np.stack([res.results[e]["out"] for e in range(N_CORES)])  # [8, 16, 1024]
    out = y[expert_indices, np.arange(T)[:, None]]  # [16, 2, 1024]
    return np.ascontiguousarray(out, dtype=np.float32)


# revision 8
# speedup vs baseline: 1.5231x; 1.5231x over previous
"""MoE ConditionalFeedForward (SwiGLU expert FFN) for 8 Trainium2 NeuronCores.

Expert-parallel: core e holds expert e's weights (host pre-packed) and
computes the full SwiGLU FFN for ALL 16 tokens densely:
    y_e = (silu(x @ w1[e].T) * (x @ w3[e].T)) @ w2[e].T        [16, 1024]
The (token, slot) -> expert routing is a pure gather done on the host:
    out[t, a] = y_{expert_indices[t, a]}[t]
Since T=16 <= 128, computing all tokens per expert costs the same PE time as
computing only the routed ones, and weight DMA dominates (memory regime).

All matmuls stream the weight matrix as the moving operand (N=512 free dim)
with the small [128, 16] activation tile stationary, so the PE streams each
weight element exactly once.

Weights are packed on the host into a [128, 22528] layout where each DMA
block is fully contiguous per partition row (16 KB lines) - 8x fewer DMA
descriptors than the naive transposed layout, which otherwise saturates the
sync engine's HWDGE descriptor generation.
"""

import os
import threading

import numpy as np

NUM_EXPERTS = 8
INTER = 2816
DIM = 1024
T = 16
A = 2
P = 128
N_CORES = 8
KD = DIM // P  # 8 contraction chunks for stage 1
KI = INTER // P  # 22 contraction chunks for stage 2
J_TILES = [(0, 512), (512, 512), (1024, 512), (1536, 512), (2048, 512), (2560, 256)]
NG2 = KI // 2  # 11 stage-2 groups of 2 chunks
WCOLS = KD * INTER  # 22528 packed columns per partition

# "f32" (float32r end-to-end), "fp16", or "bf16"
WEIGHT_MODE = os.environ.get("KERNEL_WEIGHT_MODE", "fp16")

_lock = threading.Lock()
_nc_cache = {}
LAST_RESULTS = None  # BassKernelResults of the most recent kernel() call


def _np_wdt(mode):
    if mode == "f32":
        return np.float32
    if mode == "fp16":
        return np.float16
    import ml_dtypes

    return ml_dtypes.bfloat16


def _pack_w13(w, np_wdt):
    """[INTER, DIM] -> [P, WCOLS]; block j is [P, KD, jsz] contiguous per row.

    packed[p, KD*joff + k*jsz + c] = w[joff + c, k*P + p]
    """
    blocks = []
    for joff, jsz in J_TILES:
        blk = w[joff : joff + jsz].reshape(jsz, KD, P).transpose(2, 1, 0)
        blocks.append(np.ascontiguousarray(blk, dtype=np_wdt).reshape(P, KD * jsz))
    return np.ascontiguousarray(np.concatenate(blocks, axis=1))


def _pack_w2(w2, np_wdt):
    """[DIM, INTER] -> [P, WCOLS]; group g is [P, 2, DIM] contiguous per row.

    packed[p, g*2*DIM + k*DIM + c] = w2[c, (2g + k)*P + p]
    """
    w2t = w2.T  # [INTER, DIM]
    groups = []
    for g in range(NG2):
        grp = w2t[g * 2 * P : (g + 1) * 2 * P].reshape(2, P, DIM).transpose(1, 0, 2)
        groups.append(np.ascontiguousarray(grp, dtype=np_wdt).reshape(P, 2 * DIM))
    return np.ascontiguousarray(np.concatenate(groups, axis=1))


def _build_nc(mode: str):
    import concourse.bass as bass
    import concourse.tile as tile
    from concourse import bacc, mybir

    f32 = mybir.dt.float32
    if mode == "f32":
        # float32r: same 4-byte layout as f32, single-pass PE matmul. The BIR
        # verifier requires matmul operands *declared* f32r end-to-end.
        wdt = mybir.dt.float32r
    elif mode == "fp16":
        wdt = mybir.dt.float16
    else:
        wdt = mybir.dt.bfloat16

    # Bacc (not plain Bass): its compile() runs move_matmul_waits_to_ldweights
    # + generate_event_semaphores, required because fp32r matmuls lower to a
    # self-loading LDWEIGHTS struct with a single sync-wait slot.
    nc = bacc.Bacc()
    xt_d = nc.declare_dram_parameter("xt", [P, KD * T], wdt, isOutput=False)
    w1_d = nc.declare_dram_parameter("w1p", [P, WCOLS], wdt, isOutput=False)
    w3_d = nc.declare_dram_parameter("w3p", [P, WCOLS], wdt, isOutput=False)
    w2_d = nc.declare_dram_parameter("w2p", [P, WCOLS], wdt, isOutput=False)
    eye_d = nc.declare_dram_parameter("eye", [T, T], f32, isOutput=False)
    out_d = nc.declare_dram_parameter("out", [T, DIM], f32, isOutput=True)

    with tile.TileContext(nc) as tc:
        with (
            tc.tile_pool(name="const", bufs=1) as cpool,
            tc.tile_pool(name="w1pool", bufs=3) as w1p,
            tc.tile_pool(name="w3pool", bufs=3) as w3p,
            tc.tile_pool(name="w2pool", bufs=3) as w2p,
            tc.tile_pool(name="hp", bufs=1) as hp,
            tc.tile_pool(name="ep", bufs=3) as ep,
            tc.tile_pool(name="outp", bufs=1) as outp,
            tc.tile_pool(name="ps1", bufs=4, space="PSUM") as ps1p,
            tc.tile_pool(name="pst", bufs=2, space="PSUM") as pstp,
            tc.tile_pool(name="pso", bufs=1, space="PSUM") as psop,
        ):
            xt_sb = cpool.tile([P, KD, T], wdt)
            nc.sync.dma_start(xt_sb[:], xt_d.rearrange("p (k t) -> p k t", k=KD))
            eye_sb = cpool.tile([T, T], f32)
            nc.sync.dma_start(eye_sb[:], eye_d[:])

            h_sb = hp.tile([T, INTER], f32)
            hT_sb = hp.tile([P, KI * T], wdt)

            # ---- stage 1: x1 = x@w1.T, x3 = x@w3.T, h = silu(x1)*x3 ----
            for joff, jsz in J_TILES:
                col0 = KD * joff
                w1b = w1p.tile([P, KD, jsz], wdt, tag="w1b")
                nc.sync.dma_start(
                    w1b[:],
                    w1_d[:, col0 : col0 + KD * jsz].rearrange("p (k c) -> p k c", k=KD),
                )
                w3b = w3p.tile([P, KD, jsz], wdt, tag="w3b")
                nc.sync.dma_start(
                    w3b[:],
                    w3_d[:, col0 : col0 + KD * jsz].rearrange("p (k c) -> p k c", k=KD),
                )

                ps1 = ps1p.tile([T, jsz], f32, tag="psa")
                ps3 = ps1p.tile([T, jsz], f32, tag="psa")
                for k in range(KD):
                    lhs = xt_sb[:, k, :]
                    nc.tensor.matmul(
                        ps1[:], lhs, w1b[:, k, :],
                        start=(k == 0), stop=(k == KD - 1),
                    )
                    nc.tensor.matmul(
                        ps3[:], lhs, w3b[:, k, :],
                        start=(k == 0), stop=(k == KD - 1),
                    )

                # silu(x1)*x3 = x1*sigmoid(x1)*x3 (no Silu LUT on trn2 ACT)
                sg = ep.tile([T, jsz], f32, tag="sg")
                nc.scalar.activation(
                    sg[:], ps1[:], mybir.ActivationFunctionType.Sigmoid
                )
                tmp = ep.tile([T, jsz], f32, tag="tmp")
                nc.vector.tensor_mul(tmp[:], sg[:], ps3[:])
                nc.vector.tensor_mul(h_sb[:, joff : joff + jsz], tmp[:], ps1[:])

                # transpose the freshly finished h columns into [INTER, T]
                for c in range(joff // P, (joff + jsz) // P):
                    pt = pstp.tile([P, T], f32, tag="pt")
                    nc.tensor.transpose(
                        pt[:], h_sb[:, c * P : (c + 1) * P], eye_sb[:]
                    )
                    nc.vector.tensor_copy(hT_sb[:, c * T : (c + 1) * T], pt[:])

            # ---- stage 2: out = h @ w2.T ----
            pso0 = psop.tile([T, 512], f32)
            pso1 = psop.tile([T, 512], f32)
            psos = [pso0, pso1]
            for g in range(NG2):
                col0 = g * 2 * DIM
                w2b = w2p.tile([P, 2, DIM], wdt, tag="w2b")
                nc.sync.dma_start(
                    w2b[:],
                    w2_d[:, col0 : col0 + 2 * DIM].rearrange("p (k c) -> p k c", k=2),
                )
                for k in range(2):
                    ic = 2 * g + k
                    lhs = hT_sb[:, ic * T : (ic + 1) * T]
                    for dd in range(2):
                        nc.tensor.matmul(
                            psos[dd][:], lhs, w2b[:, k, dd * 512 : (dd + 1) * 512],
                            start=(ic == 0), stop=(ic == KI - 1),
                        )

            out_sb = outp.tile([T, DIM], f32)
            nc.vector.tensor_copy(out_sb[:, 0:512], pso0[:])
            nc.vector.tensor_copy(out_sb[:, 512:1024], pso1[:])
            nc.sync.dma_start(out_d[:], out_sb[:])

    nc.compile()
    return nc


def _get_nc(mode: str):
    with _lock:
        if mode not in _nc_cache:
            _nc_cache[mode] = _build_nc(mode)
        return _nc_cache[mode]


def kernel(**inputs: np.ndarray) -> np.ndarray:
    global LAST_RESULTS
    from concourse.bass_utils import run_bass_kernel_spmd

    mode = WEIGHT_MODE
    x = np.asarray(inputs["x"], dtype=np.float32)
    expert_indices = np.asarray(inputs["expert_indices"]).astype(np.int64)
    w1 = np.asarray(inputs["w1"], dtype=np.float32)
    w2 = np.asarray(inputs["w2"], dtype=np.float32)
    w3 = np.asarray(inputs["w3"], dtype=np.float32)

    np_wdt = _np_wdt(mode)

    # x.T laid out [128 partitions, 8 k-chunks * 16 tokens]
    xt = np.ascontiguousarray(
        x.T.reshape(KD, P, T).transpose(1, 0, 2), dtype=np_wdt
    ).reshape(P, KD * T)
    eye = np.eye(T, dtype=np.float32)

    in_maps = []
    for e in range(N_CORES):
        in_maps.append(
            {
                "xt": xt,
                "w1p": _pack_w13(w1[e], np_wdt),
                "w3p": _pack_w13(w3[e], np_wdt),
                "w2p": _pack_w2(w2[e], np_wdt),
                "eye": eye,
            }
        )

    nc = _get_nc(mode)
    res = run_bass_kernel_spmd(nc, in_maps, core_ids=list(range(N_CORES)))
    LAST_RESULTS = res

    y = np.stack([res.results[e]["out"] for e in range(N_CORES)])  # [8, 16, 1024]
    out = y[expert_indices, np.arange(T)[:, None]]  # [16, 2, 1024]
    return np.ascontiguousarray(out, dtype=np.float32)
